# revision 49
# baseline (speedup 1.0000x reference)
"""Trainium2 Bass kernel for BondMessagePassing (chemprop-style D-MPNN).

Pipeline (host indexing precomputed; device work is matmuls + bulk DMA):
  - Edges sorted by dst ("slots"). Position t handles edge f(t) =
    rev[sigma(j(t))] whose src equals dst[sigma(j(t))], so the node gather
    is a host-built 0/1 expansion matmul (E) and the scatter-sum a
    selection matmul (S).  H[rev] at position t is the row gathered at t.
  - Rows are shipped post-Wh (linearity) to the core/slot that consumes
    them next layer via one AllToAll per ring.
  - The row permutation (stream order -> send buckets -> stream order) is
    done with dma_scatter_add (into zeroed buffers) on the send side and
    one dma_gather per block on the recv side -- a handful of SWDGE calls
    per block instead of per-128-row indirect DMAs.
  - SPMD: one instruction stream for all 8 cores.  Per-(ring,bin)
    subsection sizes are cross-core maxima so the stream is uniform.
"""
import sys
sys.path.insert(0, "/opt/trn_rl_repo")
import os as _os
import numpy as np
import ml_dtypes

import concourse.bass as bass
import concourse.mybir as mybir
import concourse.tile as tile
from concourse import bacc

P = 128
NCORES = 8
HID = 128
NODE_F = 128
BOND_F = 16
DEPTH = 3
SPG = int(_os.environ.get("KNOB_SPG", "2048"))  # max slots per bin
BLK = 2               # bins per block (gather/scatter unit)
import os as _os
NRING = int(_os.environ.get("KNOB_NRING", "4"))  # A2A rings = dest-group blocks

BF16 = ml_dtypes.bfloat16
FP8 = ml_dtypes.float8_e4m3


# ----------------------------------------------------------------------------
# host-side graph preprocessing
# ----------------------------------------------------------------------------

def prep(x, edge_attr, edge_index, rev_edge_index, W_i, b_i, W_h, b_h, W_o, b_o):
    N, E = x.shape[0], edge_attr.shape[0]
    src = np.asarray(edge_index[0], dtype=np.int64)
    dst = np.asarray(edge_index[1], dtype=np.int64)
    rev = np.asarray(rev_edge_index, dtype=np.int64)
    assert np.array_equal(src[rev], dst) and np.array_equal(dst[rev], src)

    sigma = np.argsort(dst, kind="stable")          # slot -> edge
    slot_of = np.empty(E, dtype=np.int64)
    slot_of[sigma] = np.arange(E)
    deg = np.bincount(dst, minlength=N)
    node_ptr = np.concatenate([[0], np.cumsum(deg)])  # node -> first slot

    # global bins (consecutive nodes, <=128 nodes, <=SPG slots), then deal by
    # size rank to cores so the g-th bin of every core has a similar slot
    # count -- shrinks the cross-core-max chunk padding of SPMD subranges
    gbins, n = [], 0
    while n < N:
        n0, s0 = n, node_ptr[n]
        while n < N and n - n0 < P and node_ptr[n + 1] - s0 <= SPG:
            n += 1
        assert n > n0, f"node {n0} degree {deg[n0]} exceeds {SPG}"
        gbins.append((n0, n - n0, int(s0), int(node_ptr[n] - s0)))
    order = np.argsort([-bb[3] for bb in gbins], kind="stable")
    bins = [[] for _ in range(NCORES)]  # per core: (n0, ncount, s0, scount)
    for rank, oi in enumerate(order):
        cc = rank % NCORES
        if (rank // NCORES) % 2:
            cc = NCORES - 1 - cc                  # snake deal
        bins[cc].append(gbins[int(oi)])
    G = max(len(b) for b in bins)
    NB = (G + BLK - 1) // BLK
    G = NB * BLK
    for bl in bins:
        while bl and len(bl) < G:
            bl.append((0, 0, 0, 0))
        while len(bl) < G:
            bl.append((0, 0, 0, 0))

    rb = np.array([(b * NRING) // NB for b in range(NB)])  # ring of block

    # per-slot lookups (global slot id -> owner core, bin, node-in-bin)
    core_of_slot = np.empty(E, dtype=np.int32)
    bin_of_slot = np.empty(E, dtype=np.int32)     # global bin index g
    nidx_of_slot = np.empty(E, dtype=np.int32)    # node index within bin
    for c in range(NCORES):
        for g, (n0, ncnt, s0, scnt) in enumerate(bins[c]):
            if not scnt:
                continue
            sl = slice(s0, s0 + scnt)
            core_of_slot[sl] = c
            bin_of_slot[sl] = g
            nidx_of_slot[sl] = dst[sigma[sl]] - n0

    tau = slot_of[rev[sigma]]                      # slot -> dest slot (involution)
    rho_of_slot = rb[bin_of_slot[tau] // BLK]      # ring carrying slot's produced row
    dcore_of_slot = core_of_slot[tau]              # dest core of slot's produced row

    # Pair-packed scatter: each SWDGE element carries 2 rows (512B) living in
    # 2 adjacent chunks at the same partition.  Both rows of a pair go to the
    # same (ring, dest) bucket at consecutive (even-based) slots.  Pairs are
    # same-bin, so each (b, r, i) subrange gets an even chunk count sized for
    # the worst core's pair demand:  sub = 2*ceil(max_c pairs_c / 128).
    cnt = np.zeros((NCORES, NB, NRING, BLK, NCORES), dtype=np.int64)
    for c in range(NCORES):
        for g, (n0, ncnt, s0, scnt) in enumerate(bins[c]):
            if not scnt:
                continue
            sl = slice(s0, s0 + scnt)
            rr = rho_of_slot[sl]
            dd = dcore_of_slot[sl]
            b, i = g // BLK, g % BLK
            np.add.at(cnt[c, b], (rr, np.full(scnt, i), dd), 1)
    npair_need = np.ceil(cnt / 2).astype(np.int64).sum(axis=4)  # [C,NB,NR,BLK]
    sub = 2 * np.ceil(npair_need.max(axis=0) / P).astype(np.int64)  # even chunks
    # pad each block's chunk total to a multiple of 4 (512-col pm windows)
    CB = sub.reshape(NB, -1).sum(axis=1)                  # chunks per block
    padc = (-CB) % 4
    CB = CB + padc                                        # includes pad chunks
    off = np.concatenate([[0], np.cumsum(CB)])            # block col offset, chunks
    TC = int(off[-1]) * P                                 # total positions per core

    # subsection start chunk within block: order (r, i)
    substart = np.zeros((NB, NRING, BLK), dtype=np.int64)
    for b in range(NB):
        s = 0
        for r in range(NRING):
            for i in range(BLK):
                substart[b, r, i] = s
                s += sub[b, r, i]
    # scatter section chunk ranges per (block, ring)
    sec0 = np.zeros((NB, NRING), dtype=np.int64)
    sec1 = np.zeros((NB, NRING), dtype=np.int64)
    for b in range(NB):
        for r in range(NRING):
            sec0[b, r] = substart[b, r, 0]
            sec1[b, r] = substart[b, r, BLK - 1] + sub[b, r, BLK - 1]

    # bucket sizing in pair units: per (ring, src core, dest core)
    cnt2p = np.zeros((NRING, NCORES, NCORES), dtype=np.int64)
    for c in range(NCORES):
        for b in range(NB):
            for r in range(NRING):
                for i in range(BLK):
                    cnt2p[r, c] += np.ceil(cnt[c, b, r, i] / 2).astype(np.int64)
    B_half = int(np.ceil((cnt2p.max() + 1) / 16) * 16)     # pairs per bucket
    B_pad = 2 * B_half                                     # rows per bucket
    B_pad = max(B_pad, int(_os.environ.get("KNOB_BPAD", "0")))
    B_half = B_pad // 2
    assert 8 * B_pad < 65536 and 8 * B_half < 32768, \
        f"B_pad {B_pad} too big; raise NRING"

    # position assignment + q allocation.  Within a (b, r, i) subrange the
    # columns hold pair-elements (p, kq) -> positions at chunks (2kq, 2kq+1),
    # partition p.  Element linear index m = kq*128 + p.  Each dest d gets
    # ceil(n_d/2) consecutive elements; odd groups pad with a dummy row.
    posslot = np.full((NCORES, TC), -1, dtype=np.int64)
    pos_of_slot = np.full(E, -1, dtype=np.int64)
    sidx_lin = np.full((NCORES, TC), -1, dtype=np.int64)   # row-granular (emulate)
    # element idx value per (c, element-slot); trash = own-core last pair
    eidx = np.zeros((NCORES, TC // 2), dtype=np.int32)     # one entry per pair
    for c in range(NCORES):
        eidx[c, :] = (c + 1) * B_half - 1
    recvrow_of_slot = np.full(E, -1, dtype=np.int64)       # ring-relative recv row
    counters = np.zeros((NRING, NCORES, NCORES), dtype=np.int64)  # pair units
    for c in range(NCORES):
        for g, (n0, ncnt, s0, scnt) in enumerate(bins[c]):
            if not scnt:
                continue
            b, i = g // BLK, g % BLK
            sl = np.arange(s0, s0 + scnt)
            rr = rho_of_slot[sl]
            dd = dcore_of_slot[sl]
            for r in range(NRING):
                base_chunk = off[b] + substart[b, r, i]    # abs chunk of subrange
                sec_chunk = off[b] + sec0[b, r]            # abs chunk of section
                nch_i = int(sub[b, r, i])
                m0 = 0                                     # element slot within subrange
                for d in range(NCORES):
                    js = sl[(rr == r) & (dd == d)]
                    npair = (len(js) + 1) // 2
                    if npair == 0:
                        continue
                    q0 = counters[r, c, d]
                    counters[r, c, d] += npair
                    # element slots m0..m0+npair-1 (linear over subrange)
                    mm = m0 + np.arange(npair)
                    m0 += npair
                    kq = mm // P
                    pp = mm % P
                    # element index value (pair units, whole ring buffer)
                    ev = d * B_half + (q0 + np.arange(npair))
                    # element slot within the *section* for eidx layout
                    kq_sec = kq + (base_chunk - sec_chunk) // 2
                    esl = (off[b] * P + sec0[b, r] * P) // 2 + kq_sec * P + pp
                    eidx[c, esl] = ev.astype(np.int32)
                    # positions of the two rows of each pair
                    t1 = (base_chunk + 2 * kq) * P + pp
                    t2 = (base_chunk + 2 * kq + 1) * P + pp
                    bq = 2 * (q0 + np.arange(npair))       # within-bucket row
                    j1 = js[0::2]
                    j2 = js[1::2]
                    posslot[c, t1[:len(j1)]] = j1
                    pos_of_slot[j1] = t1[:len(j1)]
                    sidx_lin[c, t1] = d * B_pad + bq
                    recvrow_of_slot[tau[j1]] = c * B_pad + bq[:len(j1)]
                    posslot[c, t2[:len(j2)]] = j2
                    pos_of_slot[j2] = t2[:len(j2)]
                    sidx_lin[c, t2] = d * B_pad + bq + 1
                    recvrow_of_slot[tau[j2]] = c * B_pad + bq[:len(j2)] + 1
                assert m0 <= nch_i * P // 2, (c, g, r, m0, nch_i)
    assert (pos_of_slot[core_of_slot >= 0] >= 0).all()
    # rows of trash/unused positions target the own-core trash rows
    sidx_lin[sidx_lin < 0] = 0  # unused by emulate (only real positions read)
    for c in range(NCORES):
        real = posslot[c] >= 0
        sidx_lin[c, ~real] = (c + 1) * B_pad - 1
    gidx_lin = np.zeros((NCORES, TC), dtype=np.int32)
    for c in range(NCORES):
        real = np.where(posslot[c] >= 0)[0]
        gidx_lin[c, real] = recvrow_of_slot[posslot[c][real]].astype(np.int32)
    assert gidx_lin.max() < 8 * B_pad and gidx_lin.min() >= 0

    # S [128, TC] fp8 (col = chunk*128 + node-in-bin), E [128, TC] fp8
    # (partition = node-in-bin, col = position); eaT [BOND_F, TC]
    S = np.zeros((NCORES, P, TC), dtype=np.float32)
    Em = np.zeros((NCORES, P, TC), dtype=np.float32)
    eaT = np.zeros((NCORES, BOND_F, TC), dtype=np.float32)
    for c in range(NCORES):
        real = np.where(posslot[c] >= 0)[0]
        js = posslot[c][real]
        nn = nidx_of_slot[js]
        pp = real % P
        kk = real // P
        S[c][pp, kk * P + nn] = 1.0
        Em[c][nn, real] = 1.0
        f = rev[sigma[js]]
        eaT[c][:, real] = edge_attr[f].T

    # chunk -> bin-in-block map and E-matmul segments per 512 window
    kbin = []          # per block: list (len CB[b]) of bin-in-block or -1 pad
    for b in range(NB):
        kb = -np.ones(CB[b], dtype=np.int64)
        for r in range(NRING):
            for i in range(BLK):
                a = substart[b, r, i]
                kb[a:a + sub[b, r, i]] = i
        kbin.append(kb)

    # idx tensors: 16-partition wrap replicated to 128 partitions
    def wrap16(lin):
        # lin [L] int -> [128, L//16] with tile[16m+p, s] = lin[s*16+p]
        L = len(lin)
        assert L % 16 == 0
        w = np.asarray(lin, dtype=np.int16).reshape(L // 16, 16).T  # [16, L/16]
        return np.tile(w, (8, 1))

    gidx16 = np.zeros((NCORES, P, TC // 16), dtype=np.int16)
    for c in range(NCORES):
        gidx16[c] = wrap16(gidx_lin[c])
    # scatter idx (pair-element granular): per (block, ring) section
    scol = np.zeros((NB, NRING), dtype=np.int64)   # col offset (in idx cols)
    sw = 0
    for b in range(NB):
        for r in range(NRING):
            scol[b, r] = sw
            sw += (sec1[b, r] - sec0[b, r]) * P // 32   # elems/16 per section
    sidx16 = np.zeros((NCORES, P, max(sw, 16)), dtype=np.int16)
    for c in range(NCORES):
        for b in range(NB):
            for r in range(NRING):
                e0 = (off[b] + sec0[b, r]) * P // 2
                e1 = (off[b] + sec1[b, r]) * P // 2
                if e1 > e0:
                    w = wrap16(eidx[c, e0:e1])
                    sidx16[c][:, scol[b, r]:scol[b, r] + (e1 - e0) // 16] = w

    # node-level arrays (padded per bin)
    x_pad = np.zeros((NCORES, G * P, NODE_F), dtype=np.float32)
    for c in range(NCORES):
        for g, (n0, ncnt, _, _) in enumerate(bins[c]):
            if ncnt:
                x_pad[c, g * P: g * P + ncnt] = x[n0:n0 + ncnt]
    W_i_x = W_i[:, :NODE_F]
    xW_pad = np.einsum("cnf,hf->cnh", x_pad, W_i_x).astype(np.float32)

    meta = dict(N=N, E=E, G=G, NB=NB, B_pad=B_pad, TC=TC, bins=bins,
                CB=CB.tolist(), off=off.tolist(), rb=rb.tolist(),
                sub=sub, substart=substart, sec0=sec0, sec1=sec1,
                scol=scol, kbin=kbin, sw=sw,
                posslot=posslot, sidx_lin=sidx_lin, gidx_lin=gidx_lin,
                rho=rho_of_slot, tau=tau)
    percore = []
    for c in range(NCORES):
        percore.append({
            "gidx16": gidx16[c],
            "sidx16": sidx16[c] if sw else np.zeros((P, 16), np.int16),
            "S": S[c].astype(FP8),
            "E": Em[c].astype(FP8),
            "eaT": eaT[c].astype(BF16),
            "x_pad": x_pad[c].astype(np.float32),
            "xT_pad": x_pad[c].T.copy().astype(BF16),
            "xW_pad": xW_pad[c].astype(BF16),
            "WieT": W_i[:, NODE_F:].T.copy().astype(BF16),
            "WhT": W_h.T.copy().astype(BF16),
            "WoxT": W_o[:, :NODE_F].T.copy().astype(BF16),
            "WoMT": W_o[:, NODE_F:].T.copy().astype(BF16),
            "negI": (-np.eye(P)).astype(BF16),
            "Ident": np.eye(P).astype(np.float32),
            "IdentB": np.eye(P).astype(BF16),
            "b_i": b_i.reshape(P, 1).astype(np.float32),
            "b_h": b_h.reshape(P, 1).astype(np.float32),
            "b_o_row": b_o.reshape(1, P).astype(BF16),
        })
    return meta, percore


# ----------------------------------------------------------------------------
# numpy emulation of the device pipeline (indexing validation)
# ----------------------------------------------------------------------------

def emulate(meta, percore, inputs):
    x = np.asarray(inputs["x"], np.float32)
    ea = np.asarray(inputs["edge_attr"], np.float32)
    W_i = np.asarray(inputs["W_i"], np.float32)
    b_i = np.asarray(inputs["b_i"], np.float32)
    W_h = np.asarray(inputs["W_h"], np.float32)
    b_h = np.asarray(inputs["b_h"], np.float32)
    W_o = np.asarray(inputs["W_o"], np.float32)
    b_o = np.asarray(inputs["b_o"], np.float32)
    src = np.asarray(inputs["edge_index"][0], np.int64)
    dst = np.asarray(inputs["edge_index"][1], np.int64)
    rev = np.asarray(inputs["rev_edge_index"], np.int64)

    TC, NB, B_pad, G = meta["TC"], meta["NB"], meta["B_pad"], meta["G"]
    posslot = meta["posslot"]
    sidx_lin, gidx_lin = meta["sidx_lin"], meta["gidx_lin"]
    rho = meta["rho"]
    off = meta["off"]
    rb = meta["rb"]
    bins = meta["bins"]

    sigma = np.argsort(dst, kind="stable")

    # per-core position arrays
    h0b = np.zeros((NCORES, TC, HID), np.float32)
    realm = posslot >= 0
    for c in range(NCORES):
        js = posslot[c][realm[c]]
        f = rev[sigma[js]]
        h0b[c][realm[c]] = (x[src[f]] @ W_i[:, :NODE_F].T
                            + ea[f] @ W_i[:, NODE_F:].T + b_i)
    H = np.maximum(h0b, 0.0)

    def ship(rows_out):
        # rows_out [NCORES, TC, HID] -> gathered rows [NCORES, TC, HID]
        send = np.zeros((NRING, NCORES, 8 * B_pad + 16, HID), np.float32)
        for c in range(NCORES):
            js = posslot[c][realm[c]]
            rr = rho[js]
            si = sidx_lin[c][realm[c]]
            send[rr, c, si] = rows_out[c][realm[c]]
        # A2A per ring
        recv = np.zeros((NRING, NCORES, 8 * B_pad, HID), np.float32)
        for r in range(NRING):
            for d in range(NCORES):
                for cc in range(NCORES):
                    recv[r, d, cc * B_pad:(cc + 1) * B_pad] = \
                        send[r, cc, d * B_pad:(d + 1) * B_pad]
        rows_in = np.zeros((NCORES, TC, HID), np.float32)
        for c in range(NCORES):
            for b in range(NB):
                t0, t1 = off[b] * P, off[b + 1] * P
                rows_in[c, t0:t1] = recv[rb[b], c][gidx_lin[c, t0:t1]]
        return rows_in

    def aggregate(rows_in):
        # agg per bin node from S; also M = agg[node(t)] - rows_in[t]
        agg = np.zeros((NCORES, G, P, HID), np.float32)
        for c in range(NCORES):
            js = posslot[c][realm[c]]
            g = np.zeros(len(js), np.int64)
            # bin of slot
            for gi, (n0, ncnt, s0, scnt) in enumerate(bins[c]):
                if scnt:
                    g[(js >= s0) & (js < s0 + scnt)] = gi
            nn = dst[sigma[js]] - np.array(
                [bins[c][gi][0] for gi in g])
            np.add.at(agg[c], (g, nn), rows_in[c][realm[c]])
        return agg

    for it in range(1, DEPTH):
        rows_out = np.zeros((NCORES, TC, HID), np.float32)
        rhs = W_h.T if it < DEPTH - 1 or True else None
        for c in range(NCORES):
            rows_out[c] = H[c] @ W_h.T
        rows_in = ship(rows_out)
        agg = aggregate(rows_in)
        for c in range(NCORES):
            js = posslot[c][realm[c]]
            g = np.zeros(len(js), np.int64)
            for gi, (n0, ncnt, s0, scnt) in enumerate(bins[c]):
                if scnt:
                    g[(js >= s0) & (js < s0 + scnt)] = gi
            nn = dst[sigma[js]] - np.array([bins[c][gi][0] for gi in g])
            M = agg[c][g, nn] - rows_in[c][realm[c]]
            Hc = np.zeros((TC, HID), np.float32)
            Hc[realm[c]] = np.maximum(h0b[c][realm[c]] + M + b_h, 0.0)
            H[c] = Hc

    # final aggregation of H itself (shipped with identity)
    rows_in = ship(H)
    agg = aggregate(rows_in)

    outs = []
    for c in range(NCORES):
        out = np.zeros((G * P, HID), np.float32)
        for gi, (n0, ncnt, s0, scnt) in enumerate(bins[c]):
            if not ncnt:
                continue
            a = agg[c][gi]
            mask = (a.sum(axis=1) == 0.0).astype(np.float32)[:, None]
            xg = np.zeros((P, NODE_F), np.float32)
            xg[:ncnt] = x[n0:n0 + ncnt]
            M = a + mask * xg
            o = np.concatenate([xg, M], axis=1) @ W_o.T + b_o
            out[gi * P:(gi + 1) * P] = np.maximum(o, 0.0)
        outs.append(out)
    return outs


def assemble(meta, outs):
    N = meta["N"]
    full = np.zeros((N, HID), np.float32)
    for c in range(NCORES):
        for g, (n0, ncnt, _, _) in enumerate(meta["bins"][c]):
            if ncnt:
                full[n0:n0 + ncnt] = outs[c][g * P: g * P + ncnt]
    return full


# ----------------------------------------------------------------------------
# bass kernel
# ----------------------------------------------------------------------------

def build_nc(meta):
    DT = mybir.dt
    NB, B_pad, TC, G = meta["NB"], meta["B_pad"], meta["TC"], meta["G"]
    CB, off, rb = meta["CB"], meta["off"], meta["rb"]
    sec0, sec1, scol, kbin = meta["sec0"], meta["sec1"], meta["scol"], meta["kbin"]
    sw = max(meta["sw"], 16)
    W = 8 * B_pad

    nc = bacc.Bacc("TRN2", target_bir_lowering=False, debug=False,
                   num_devices=NCORES,
                   num_swdge_queues=int(_os.environ.get("KNOB_NSWQ", "2")))
    t = {}
    def inp(name, shape, dt):
        t[name] = nc.dram_tensor(name, shape, dt, kind="ExternalInput")
        return t[name]

    inp("gidx16", [P, TC // 16], DT.int16)
    inp("sidx16", [P, sw], DT.int16)
    inp("S", [P, TC], DT.float8e4)
    inp("E", [P, TC], DT.float8e4)
    inp("eaT", [BOND_F, TC], DT.bfloat16)
    inp("x_pad", [G * P, NODE_F], DT.float32)
    inp("xT_pad", [P, G * P], DT.bfloat16)
    inp("xW_pad", [G * P, HID], DT.bfloat16)
    inp("WieT", [BOND_F, HID], DT.bfloat16)
    inp("WhT", [P, P], DT.bfloat16)
    inp("WoxT", [P, P], DT.bfloat16)
    inp("WoMT", [P, P], DT.bfloat16)
    inp("negI", [P, P], DT.bfloat16)
    inp("Ident", [P, P], DT.float32)
    inp("IdentB", [P, P], DT.bfloat16)
    inp("b_i", [P, 1], DT.float32)
    inp("b_h", [P, 1], DT.float32)
    inp("b_o_row", [1, P], DT.bfloat16)
    out_pad = nc.dram_tensor("out_pad", [G * P, HID], DT.float32,
                             kind="ExternalOutput")

    h0t = nc.dram_tensor("h0t", [P, TC], DT.bfloat16)
    # pair-element layout: row = 2 edge-rows (512B)
    sends = [[nc.dram_tensor(f"send{s}_{r}", [W // 2, 2 * HID], DT.bfloat16)
              for r in range(NRING)] for s in range(2)]
    recvs = [nc.dram_tensor(f"recv{r}", [W // 2, 2 * HID], DT.bfloat16)
             for r in range(NRING)]

    AF = mybir.ActivationFunctionType
    OP = mybir.AluOpType
    RG = [list(range(NCORES))]
    NOSC = bool(_os.environ.get("KNOB_NOSC"))
    NOGA = bool(_os.environ.get("KNOB_NOGA"))
    NOCC = bool(_os.environ.get("KNOB_NOCC"))
    NOZ = bool(_os.environ.get("KNOB_NOZ"))
    GSPLIT = int(_os.environ.get("KNOB_GSPLIT", "8"))
    SP = not _os.environ.get("KNOB_NOSP")

    with tile.TileContext(nc) as tc:
        with (
            tc.tile_pool(name="sb", bufs=2) as sb,
            tc.tile_pool(name="sm", bufs=int(_os.environ.get("KNOB_SMBUFS",
                                                             "3"))) as sm,
            tc.tile_pool(name="smh", bufs=2) as smh,
            tc.tile_pool(name="cst", bufs=1) as cst,
            tc.tile_pool(name="ps", bufs=3, space="PSUM") as ps,
            tc.tile_pool(name="ps1", bufs=3, space="PSUM") as ps1,
            tc.tile_pool(name="psA", bufs=1, space="PSUM") as psA,
        ):
            # resident constants
            c_wiet = cst.tile([BOND_F, HID], DT.bfloat16)
            nc.sync.dma_start(c_wiet[:], t["WieT"][:])
            c_wht = cst.tile([P, P], DT.bfloat16)
            nc.sync.dma_start(c_wht[:], t["WhT"][:])
            c_woxt = cst.tile([P, P], DT.bfloat16)
            nc.sync.dma_start(c_woxt[:], t["WoxT"][:])
            c_womt = cst.tile([P, P], DT.bfloat16)
            nc.sync.dma_start(c_womt[:], t["WoMT"][:])
            c_negi = cst.tile([P, P], DT.bfloat16)
            nc.sync.dma_start(c_negi[:], t["negI"][:])
            c_id = cst.tile([P, P], DT.float32)
            nc.sync.dma_start(c_id[:], t["Ident"][:])
            c_idb = cst.tile([P, P], DT.bfloat16)
            nc.sync.dma_start(c_idb[:], t["IdentB"][:])
            c_bi = cst.tile([P, 1], DT.float32)
            nc.sync.dma_start(c_bi[:], t["b_i"][:])
            c_bh = cst.tile([P, 1], DT.float32)
            nc.sync.dma_start(c_bh[:], t["b_h"][:])
            c_bibh = cst.tile([P, 1], DT.float32)
            nc.vector.tensor_tensor(c_bibh[:], c_bi[:], c_bh[:], op=OP.add)
            c_nbh = cst.tile([P, 1], DT.float32)
            nc.vector.tensor_scalar(c_nbh[:], c_bh[:], -1.0, None, op0=OP.mult)
            c_bo = cst.tile([1, P], DT.bfloat16)
            nc.sync.dma_start(c_bo[:], t["b_o_row"][:])
            c_one = cst.tile([1, P], DT.bfloat16)
            nc.vector.memset(c_one[:], 1.0)
            c_xt = cst.tile([P, G * P], DT.bfloat16)
            nc.sync.dma_start(c_xt[:], t["xT_pad"][:])
            c_zero = cst.tile([P, 2048], DT.bfloat16)
            nc.vector.memset(c_zero[:], 0.0)
            c_zb = c_zero[:, 0:P]
            c_si = cst.tile([P, sw], DT.int16)
            nc.scalar.dma_start(c_si[:], t["sidx16"][:, :])

            def zero_send(s, r):
                if NOZ:
                    return
                for z0 in range(0, W // 2, 1024):
                    z1 = min(z0 + 1024, W // 2)
                    k = (z1 - z0) // P
                    nc.scalar.dma_start(
                        sends[s][r].ap()[z0:z1, :]
                        .rearrange("(p k) h -> p (k h)", k=k),
                        c_zero[:, :k * 2 * P])

            def scatters(b, outrows, si_t, s):
                if NOSC:
                    return
                for r in range(NRING):
                    nchp = int(sec1[b, r] - sec0[b, r]) // 2   # pair-chunks
                    c0 = int(scol[b, r])
                    for k0 in range(0, nchp, GSPLIT):
                        k1 = min(k0 + GSPLIT, nchp)
                        L = (k1 - k0) * P
                        nc.gpsimd.dma_scatter_add(
                            sends[s][r][:],
                            outrows[:, (int(sec0[b, r]) + 2 * k0) * P
                                    :(int(sec0[b, r]) + 2 * k1) * P]
                            .rearrange("p (k h) -> p k h", h=2 * HID),
                            si_t[:, c0 + k0 * 8:c0 + k1 * 8],
                            L, L, 2 * HID, single_packet=SP)

            # initial zero of both send sets
            for s in range(2):
                for r in range(NRING):
                    zero_send(s, r)

            # ---------------- phase 0 ----------------
            for b in range(NB):
                nch = CB[b]
                cols = slice(off[b] * P, off[b + 1] * P)
                e_t = sm.tile([P, nch * P], DT.float8e4, tag="E")
                nc.scalar.dma_start(e_t[:], t["E"][:, cols])
                ea_t = sm.tile([BOND_F, nch * P], DT.bfloat16, tag="ea")
                nc.sync.dma_start(ea_t[:], t["eaT"][:, cols])
                xw = []
                for i in range(BLK):
                    g = b * BLK + i
                    xw_g = sm.tile([P, HID], DT.bfloat16, tag=f"xw{i}")
                    nc.sync.dma_start(xw_g[:], t["xW_pad"][g * P:(g + 1) * P, :])
                    xw.append(xw_g)
                h0s = sb.tile([P, nch * P], DT.bfloat16, tag="h0s")
                outrows = sb.tile([P, nch * P], DT.bfloat16, tag="or")
                kb = kbin[b]
                for s in range(nch // 4):
                    if all(int(kb[s * 4 + k]) < 0 for k in range(4)):
                        continue
                    win = slice(s * 512, (s + 1) * 512)
                    pm = ps.tile([P, 512], DT.float32, tag="pm")
                    binw = [max(int(kb[s * 4 + k]), 0) for k in range(4)]
                    if len(set(binw)) == 1:
                        nc.tensor.matmul(pm[:, :], lhsT=xw[binw[0]][:],
                                         rhs=e_t[:, win], start=True, stop=False)
                        nc.tensor.matmul(pm[:, :], lhsT=c_wiet[:],
                                         rhs=ea_t[:, win], start=False, stop=True)
                    else:
                        for k in range(4):
                            ka = s * 4 + k
                            kc = slice(k * P, (k + 1) * P)
                            kca = slice(ka * P, (ka + 1) * P)
                            nc.tensor.matmul(pm[:, kc], lhsT=xw[binw[k]][:],
                                             rhs=e_t[:, kca], start=True,
                                             stop=False)
                            nc.tensor.matmul(pm[:, kc], lhsT=c_wiet[:],
                                             rhs=ea_t[:, kca], start=False,
                                             stop=True)
                    nc.vector.tensor_scalar(h0s[:, win], pm[:], c_bibh[:], None,
                                            op0=OP.add)
                    h1 = sb.tile([P, 512], DT.bfloat16, tag="h1")
                    nc.scalar.activation(h1[:], h0s[:, win], AF.Relu,
                                         bias=c_nbh[:])
                    pw = ps1.tile([P, 512], DT.float32, tag="pw")
                    for k in range(4):
                        kc = slice(k * P, (k + 1) * P)
                        nc.tensor.matmul(pw[:, kc], lhsT=h1[:, kc],
                                         rhs=c_wht[:], start=True, stop=True)
                    nc.vector.tensor_scalar(outrows[:, win], pw[:], 0.0, None,
                                            op0=OP.add)
                nc.sync.dma_start(h0t[:, cols], h0s[:])
                scatters(b, outrows, c_si, 0)

            # ---------------- phases 1..DEPTH ----------------
            for it in range(1, DEPTH + 1):
                sprev = (it - 1) % 2
                scur = it % 2
                if not NOCC:
                    for r in range(NRING):
                        nc.gpsimd.collective_compute(
                            "AllToAll", OP.bypass, replica_groups=RG,
                            ins=[sends[sprev][r][:]],
                            outs=[recvs[r][:]])
                if it == 1:
                    # re-zero set 0 now (WAR on this phase's A2A reads) so the
                    # writes overlap phase-1 compute instead of phase-2 start
                    for r in range(NRING):
                        zero_send(0, r)
                last = it == DEPTH
                for b in range(NB):
                    nch = CB[b]
                    cols = slice(off[b] * P, off[b + 1] * P)
                    kb = kbin[b]
                    gi_t = sm.tile([P, nch * 8], DT.int16, tag="gi")
                    nc.sync.dma_start(gi_t[:],
                                      t["gidx16"][:, off[b] * 8:off[b + 1] * 8])
                    rows = sb.tile([P, nch * P], DT.bfloat16, tag="rows")
                    rnch = int(sec1[b, NRING - 1])  # skip window-pad chunks
                    if NOGA:
                        nc.vector.memset(rows[:], 0.0)
                    else:
                        for c0 in range(0, rnch, GSPLIT):
                            c1 = min(c0 + GSPLIT, rnch)
                            nc.gpsimd.dma_gather(
                                rows[:, c0 * P:c1 * P]
                                .rearrange("p (k h) -> p k h", h=HID),
                                recvs[rb[b]].ap()
                                .rearrange("w (two h) -> (w two) h", two=2),
                                gi_t[:, c0 * 8:c1 * 8],
                                (c1 - c0) * P, (c1 - c0) * P, HID,
                                single_packet=SP,
                                queue_num=1 if nc.num_swdge_queues > 1 else 0)
                    s_t = sm.tile([P, nch * P], DT.float8e4, tag="S")
                    nc.sync.dma_start(s_t[:], t["S"][:, cols])
                    # one PSUM bank (512 f32) per bin: start=True clears the
                    # has_written bits of the whole bank, so the two bins'
                    # accumulation chains must not share one
                    pa = psA.tile([P, BLK * 512], DT.float32, tag="pa")
                    pav = [pa[:, i * 512:i * 512 + HID] for i in range(BLK)]
                    kfirst = [min([k for k in range(nch) if kb[k] == i],
                                  default=-1) for i in range(BLK)]
                    klast = [max([k for k in range(nch) if kb[k] == i],
                                 default=-1) for i in range(BLK)]
                    for k in range(nch):
                        i = int(kb[k])
                        if i < 0:
                            continue
                        kc = slice(k * P, (k + 1) * P)
                        nc.tensor.matmul(pav[i],
                                         lhsT=s_t[:, kc], rhs=rows[:, kc],
                                         start=(k == kfirst[i]),
                                         stop=(k == klast[i]))
                    for i in range(BLK):
                        if kfirst[i] < 0:
                            nc.tensor.matmul(pav[i],
                                             lhsT=c_zb[:], rhs=rows[:, 0:P],
                                             start=True, stop=True)

                    if last:
                        # ---- readout ----
                        for i in range(BLK):
                            g = b * BLK + i
                            agg3 = sb.tile([P, HID], DT.float32, tag="agg3")
                            nc.scalar.activation(
                                agg3[:], pav[i], AF.Copy)
                            rsum = sb.tile([P, 1], DT.float32, tag="rsum")
                            nc.vector.tensor_reduce(rsum[:], agg3[:],
                                                    axis=mybir.AxisListType.X,
                                                    op=OP.add)
                            mask = sb.tile([P, 1], DT.float32, tag="mask")
                            nc.vector.tensor_scalar(mask[:], rsum[:], 0.0, None,
                                                    op0=OP.is_equal)
                            x_g = sb.tile([P, NODE_F], DT.float32, tag="xg")
                            nc.sync.dma_start(x_g[:],
                                              t["x_pad"][g * P:(g + 1) * P, :])
                            mx = sb.tile([P, NODE_F], DT.float32, tag="mx")
                            nc.vector.tensor_scalar(mx[:], x_g[:], mask[:],
                                                    None, op0=OP.mult)
                            Mg = sb.tile([P, HID], DT.float32, tag="Mg")
                            nc.vector.tensor_tensor(Mg[:], agg3[:], mx[:],
                                                    op=OP.add)
                            pt = ps1.tile([P, P], DT.float32, tag="pw")
                            nc.tensor.transpose(pt[:], Mg[:], c_id[:])
                            MgT = sb.tile([P, P], DT.bfloat16, tag="MgT")
                            nc.scalar.activation(MgT[:], pt[:], AF.Copy)
                            po = ps.tile([P, HID], DT.float32, tag="pm")
                            nc.tensor.matmul(
                                po[:], lhsT=c_xt[:, g * P:(g + 1) * P],
                                rhs=c_woxt[:], start=True, stop=False)
                            nc.tensor.matmul(po[:], lhsT=MgT[:], rhs=c_womt[:],
                                             start=False, stop=False)
                            nc.tensor.matmul(po[:], lhsT=c_one[:], rhs=c_bo[:],
                                             start=False, stop=True)
                            og = sb.tile([P, HID], DT.float32, tag="og")
                            nc.scalar.activation(og[:], po[:], AF.Relu)
                            nc.sync.dma_start(out_pad[g * P:(g + 1) * P, :],
                                              og[:])
                        continue

                    aggw = sb.tile([P, BLK * HID], DT.bfloat16, tag="aggw")
                    for i in range(BLK):
                        nc.scalar.activation(
                            aggw[:, i * HID:(i + 1) * HID], pav[i], AF.Copy)
                    e_t = sm.tile([P, nch * P], DT.float8e4, tag="E")
                    nc.scalar.dma_start(e_t[:], t["E"][:, cols])
                    h0_t = smh.tile([P, nch * P], DT.bfloat16, tag="h0g")
                    nc.sync.dma_start(h0_t[:], h0t[:, cols])
                    outrows = sb.tile([P, nch * P], DT.bfloat16, tag="or")
                    rhs_w = c_wht if it < DEPTH - 1 else c_idb
                    for s in range(nch // 4):
                        if all(int(kb[s * 4 + k]) < 0 for k in range(4)):
                            continue
                        win = slice(s * 512, (s + 1) * 512)
                        pm = ps.tile([P, 512], DT.float32, tag="pm")
                        binw = [max(int(kb[s * 4 + k]), 0) for k in range(4)]
                        if len(set(binw)) == 1:
                            nc.tensor.matmul(
                                pm[:, :],
                                lhsT=aggw[:, binw[0] * HID
                                          :(binw[0] + 1) * HID],
                                rhs=e_t[:, win], start=True, stop=False)
                            for k in range(4):
                                ka = s * 4 + k
                                kc = slice(k * P, (k + 1) * P)
                                kca = slice(ka * P, (ka + 1) * P)
                                nc.tensor.matmul(pm[:, kc], lhsT=rows[:, kca],
                                                 rhs=c_negi[:], start=False,
                                                 stop=True)
                        else:
                            for k in range(4):
                                ka = s * 4 + k
                                kc = slice(k * P, (k + 1) * P)
                                kca = slice(ka * P, (ka + 1) * P)
                                nc.tensor.matmul(
                                    pm[:, kc],
                                    lhsT=aggw[:, binw[k] * HID
                                              :(binw[k] + 1) * HID],
                                    rhs=e_t[:, kca], start=True, stop=False)
                                nc.tensor.matmul(pm[:, kc], lhsT=rows[:, kca],
                                                 rhs=c_negi[:], start=False,
                                                 stop=True)
                        tmp = sb.tile([P, 512], DT.bfloat16, tag="tmp")
                        nc.vector.tensor_tensor(tmp[:], pm[:], h0_t[:, win],
                                                op=OP.add)
                        hn = sb.tile([P, 512], DT.bfloat16, tag="hn")
                        nc.scalar.activation(hn[:], tmp[:], AF.Relu)
                        pw = ps1.tile([P, 512], DT.float32, tag="pw")
                        for k in range(4):
                            kc = slice(k * P, (k + 1) * P)
                            nc.tensor.matmul(pw[:, kc], lhsT=hn[:, kc],
                                             rhs=rhs_w[:], start=True,
                                             stop=True)
                        nc.vector.tensor_scalar(outrows[:, win], pw[:], 0.0,
                                                None, op0=OP.add)
                    scatters(b, outrows, c_si, scur)

    nc.compile()
    return nc


# ----------------------------------------------------------------------------
# PJRT SPMD runner (inlined; based on concourse.bass2jax.run_bass_via_pjrt)
# ----------------------------------------------------------------------------

class SpmdRunner:
    def __init__(self, nc, n_cores):
        import jax
        from jax.sharding import Mesh, PartitionSpec
        from jax.experimental.shard_map import shard_map
        from concourse.bass2jax import (
            _bass_exec_p, partition_id_tensor, install_neuronx_cc_hook)
        install_neuronx_cc_hook()
        self.jax = jax
        self.n_cores = n_cores
        in_names, out_names, out_avals, zero_outs = [], [], [], []
        partition_name = (
            nc.partition_id_tensor.name if nc.partition_id_tensor else None)
        for alloc in nc.m.functions[0].allocations:
            if not isinstance(alloc, mybir.MemoryLocationSet):
                continue
            name = alloc.memorylocations[0].name
            if alloc.kind == "ExternalInput":
                if name != partition_name:
                    in_names.append(name)
            elif alloc.kind == "ExternalOutput":
                out_names.append(name)
                shape = tuple(alloc.tensor_shape)
                dtype = mybir.dt.np(alloc.dtype)
                out_avals.append(jax.core.ShapedArray(shape, dtype))
                zero_outs.append(np.zeros(shape, dtype))
        self.in_names, self.out_names = in_names, out_names
        self.out_avals, self.zero_outs = out_avals, zero_outs
        n_params, n_outs = len(in_names), len(out_avals)
        all_in = list(in_names) + list(out_names)
        if partition_name is not None:
            all_in.append(partition_name)

        def _body(*args):
            operands = list(args)
            if partition_name is not None:
                operands.append(partition_id_tensor())
            return tuple(_bass_exec_p.bind(
                *operands, out_avals=tuple(out_avals),
                in_names=tuple(all_in), out_names=tuple(out_names),
                lowering_input_output_aliases=(),
                sim_require_finite=True, sim_require_nnan=True, nc=nc))

        devices = jax.devices()[:n_cores]
        assert len(devices) == n_cores
        self.mesh = Mesh(np.asarray(devices), ("core",))
        self.PartitionSpec = PartitionSpec
        in_specs = (PartitionSpec("core"),) * (n_params + n_outs)
        out_specs = (PartitionSpec("core"),) * len(out_names)
        self.fn = jax.jit(
            shard_map(_body, mesh=self.mesh, in_specs=in_specs,
                      out_specs=out_specs, check_rep=False),
            keep_unused=True)

    def pack(self, in_maps):
        n = self.n_cores
        concat = [np.concatenate(
            [np.asarray(in_maps[c][name]) for c in range(n)], axis=0)
            for name in self.in_names]
        for z in self.zero_outs:
            concat.append(np.zeros((n * z.shape[0], *z.shape[1:]), z.dtype))
        sharding = self.jax.sharding.NamedSharding(
            self.mesh, self.PartitionSpec("core"))
        return [self.jax.device_put(a, sharding) for a in concat]

    def run(self, packed):
        outs = self.fn(*packed)
        self.jax.block_until_ready(outs)
        return outs

    def unpack(self, outs):
        n = self.n_cores
        return [
            {name: np.asarray(outs[i]).reshape(n, *self.out_avals[i].shape)[c]
             for i, name in enumerate(self.out_names)}
            for c in range(n)]

    def time_exec(self, packed, iters=10, warmup=2):
        import time
        for _ in range(warmup):
            self.jax.block_until_ready(self.fn(*packed))
        t0 = time.perf_counter()
        outs = None
        for _ in range(iters):
            outs = self.fn(*packed)
        self.jax.block_until_ready(outs)
        t1 = time.perf_counter()
        return (t1 - t0) / iters, outs


def kernel(**inputs):
    inputs = {k: np.asarray(v) for k, v in inputs.items()}
    meta, percore = prep(**inputs)
    nc = build_nc(meta)
    r = SpmdRunner(nc, NCORES)
    packed = r.pack(percore)
    outs = r.run(packed)
    res = r.unpack(outs)
    return assemble(meta, [res[c]["out_pad"] for c in range(NCORES)])



# revision 50
# speedup vs baseline: 1.0485x; 1.0485x over previous
"""Trainium2 Bass kernel for BondMessagePassing (chemprop-style D-MPNN).

Pipeline (host indexing precomputed; device work is matmuls + bulk DMA):
  - Edges sorted by dst ("slots"). Position t handles edge f(t) =
    rev[sigma(j(t))] whose src equals dst[sigma(j(t))], so the node gather
    is a host-built 0/1 expansion matmul (E) and the scatter-sum a
    selection matmul (S).  H[rev] at position t is the row gathered at t.
  - Rows are shipped post-Wh (linearity) to the core/slot that consumes
    them next layer via one AllToAll per ring.
  - The row permutation (stream order -> send buckets -> stream order) is
    done with dma_scatter_add (into zeroed buffers) on the send side and
    one dma_gather per block on the recv side -- a handful of SWDGE calls
    per block instead of per-128-row indirect DMAs.
  - SPMD: one instruction stream for all 8 cores.  Per-(ring,bin)
    subsection sizes are cross-core maxima so the stream is uniform.
"""
import sys
sys.path.insert(0, "/opt/trn_rl_repo")
import os as _os
import numpy as np
import ml_dtypes

import concourse.bass as bass
import concourse.mybir as mybir
import concourse.tile as tile
from concourse import bacc

P = 128
NCORES = 8
HID = 128
NODE_F = 128
BOND_F = 16
DEPTH = 3
SPG = int(_os.environ.get("KNOB_SPG", "2048"))  # max slots per bin
BLK = 2               # bins per block (gather/scatter unit)
import os as _os
NRING = int(_os.environ.get("KNOB_NRING", "4"))  # A2A rings = dest-group blocks

BF16 = ml_dtypes.bfloat16
FP8 = ml_dtypes.float8_e4m3


# ----------------------------------------------------------------------------
# host-side graph preprocessing
# ----------------------------------------------------------------------------

def prep(x, edge_attr, edge_index, rev_edge_index, W_i, b_i, W_h, b_h, W_o, b_o):
    N, E = x.shape[0], edge_attr.shape[0]
    src = np.asarray(edge_index[0], dtype=np.int64)
    dst = np.asarray(edge_index[1], dtype=np.int64)
    rev = np.asarray(rev_edge_index, dtype=np.int64)
    assert np.array_equal(src[rev], dst) and np.array_equal(dst[rev], src)

    sigma = np.argsort(dst, kind="stable")          # slot -> edge
    slot_of = np.empty(E, dtype=np.int64)
    slot_of[sigma] = np.arange(E)
    deg = np.bincount(dst, minlength=N)
    node_ptr = np.concatenate([[0], np.cumsum(deg)])  # node -> first slot

    # global bins (consecutive nodes, <=128 nodes, <=SPG slots), then deal by
    # size rank to cores so the g-th bin of every core has a similar slot
    # count -- shrinks the cross-core-max chunk padding of SPMD subranges
    gbins, n = [], 0
    while n < N:
        n0, s0 = n, node_ptr[n]
        while n < N and n - n0 < P and node_ptr[n + 1] - s0 <= SPG:
            n += 1
        assert n > n0, f"node {n0} degree {deg[n0]} exceeds {SPG}"
        gbins.append((n0, n - n0, int(s0), int(node_ptr[n] - s0)))
    order = np.argsort([-bb[3] for bb in gbins], kind="stable")
    bins = [[] for _ in range(NCORES)]  # per core: (n0, ncount, s0, scount)
    for rank, oi in enumerate(order):
        cc = rank % NCORES
        if (rank // NCORES) % 2:
            cc = NCORES - 1 - cc                  # snake deal
        bins[cc].append(gbins[int(oi)])
    G = max(len(b) for b in bins)
    NB = (G + BLK - 1) // BLK
    G = NB * BLK
    for bl in bins:
        while bl and len(bl) < G:
            bl.append((0, 0, 0, 0))
        while len(bl) < G:
            bl.append((0, 0, 0, 0))

    rb = np.array([(b * NRING) // NB for b in range(NB)])  # ring of block

    # per-slot lookups (global slot id -> owner core, bin, node-in-bin)
    core_of_slot = np.empty(E, dtype=np.int32)
    bin_of_slot = np.empty(E, dtype=np.int32)     # global bin index g
    nidx_of_slot = np.empty(E, dtype=np.int32)    # node index within bin
    for c in range(NCORES):
        for g, (n0, ncnt, s0, scnt) in enumerate(bins[c]):
            if not scnt:
                continue
            sl = slice(s0, s0 + scnt)
            core_of_slot[sl] = c
            bin_of_slot[sl] = g
            nidx_of_slot[sl] = dst[sigma[sl]] - n0

    tau = slot_of[rev[sigma]]                      # slot -> dest slot (involution)
    rho_of_slot = rb[bin_of_slot[tau] // BLK]      # ring carrying slot's produced row
    dcore_of_slot = core_of_slot[tau]              # dest core of slot's produced row

    # Pair-packed scatter: each SWDGE element carries 2 rows (512B) living in
    # 2 adjacent chunks at the same partition.  Both rows of a pair go to the
    # same (ring, dest) bucket at consecutive (even-based) slots.  Pairs are
    # same-bin, so each (b, r, i) subrange gets an even chunk count sized for
    # the worst core's pair demand:  sub = 2*ceil(max_c pairs_c / 128).
    cnt = np.zeros((NCORES, NB, NRING, BLK, NCORES), dtype=np.int64)
    for c in range(NCORES):
        for g, (n0, ncnt, s0, scnt) in enumerate(bins[c]):
            if not scnt:
                continue
            sl = slice(s0, s0 + scnt)
            rr = rho_of_slot[sl]
            dd = dcore_of_slot[sl]
            b, i = g // BLK, g % BLK
            np.add.at(cnt[c, b], (rr, np.full(scnt, i), dd), 1)
    npair_need = np.ceil(cnt / 2).astype(np.int64).sum(axis=4)  # [C,NB,NR,BLK]
    sub = 2 * np.ceil(npair_need.max(axis=0) / P).astype(np.int64)  # even chunks
    # pad each block's chunk total to a multiple of 4 (512-col pm windows)
    CB = sub.reshape(NB, -1).sum(axis=1)                  # chunks per block
    padc = (-CB) % 4
    CB = CB + padc                                        # includes pad chunks
    off = np.concatenate([[0], np.cumsum(CB)])            # block col offset, chunks
    TC = int(off[-1]) * P                                 # total positions per core

    # subsection start chunk within block: order (r, i)
    substart = np.zeros((NB, NRING, BLK), dtype=np.int64)
    for b in range(NB):
        s = 0
        for r in range(NRING):
            for i in range(BLK):
                substart[b, r, i] = s
                s += sub[b, r, i]
    # scatter section chunk ranges per (block, ring)
    sec0 = np.zeros((NB, NRING), dtype=np.int64)
    sec1 = np.zeros((NB, NRING), dtype=np.int64)
    for b in range(NB):
        for r in range(NRING):
            sec0[b, r] = substart[b, r, 0]
            sec1[b, r] = substart[b, r, BLK - 1] + sub[b, r, BLK - 1]

    # bucket sizing in pair units: per (ring, src core, dest core)
    cnt2p = np.zeros((NRING, NCORES, NCORES), dtype=np.int64)
    for c in range(NCORES):
        for b in range(NB):
            for r in range(NRING):
                for i in range(BLK):
                    cnt2p[r, c] += np.ceil(cnt[c, b, r, i] / 2).astype(np.int64)
    B_half = int(np.ceil((cnt2p.max() + 1) / 16) * 16)     # pairs per bucket
    B_pad = 2 * B_half                                     # rows per bucket
    B_pad = max(B_pad, int(_os.environ.get("KNOB_BPAD", "0")))
    B_half = B_pad // 2
    assert 8 * B_pad < 65536 and 8 * B_half < 32768, \
        f"B_pad {B_pad} too big; raise NRING"

    # position assignment + q allocation.  Within a (b, r, i) subrange the
    # columns hold pair-elements (p, kq) -> positions at chunks (2kq, 2kq+1),
    # partition p.  Element linear index m = kq*128 + p.  Each dest d gets
    # ceil(n_d/2) consecutive elements; odd groups pad with a dummy row.
    posslot = np.full((NCORES, TC), -1, dtype=np.int64)
    pos_of_slot = np.full(E, -1, dtype=np.int64)
    sidx_lin = np.full((NCORES, TC), -1, dtype=np.int64)   # row-granular (emulate)
    # element idx value per (c, element-slot); trash = own-core last pair
    eidx = np.zeros((NCORES, TC // 2), dtype=np.int32)     # one entry per pair
    for c in range(NCORES):
        eidx[c, :] = (c + 1) * B_half - 1
    recvrow_of_slot = np.full(E, -1, dtype=np.int64)       # ring-relative recv row
    counters = np.zeros((NRING, NCORES, NCORES), dtype=np.int64)  # pair units
    for c in range(NCORES):
        for g, (n0, ncnt, s0, scnt) in enumerate(bins[c]):
            if not scnt:
                continue
            b, i = g // BLK, g % BLK
            sl = np.arange(s0, s0 + scnt)
            rr = rho_of_slot[sl]
            dd = dcore_of_slot[sl]
            for r in range(NRING):
                base_chunk = off[b] + substart[b, r, i]    # abs chunk of subrange
                sec_chunk = off[b] + sec0[b, r]            # abs chunk of section
                nch_i = int(sub[b, r, i])
                m0 = 0                                     # element slot within subrange
                for d in range(NCORES):
                    js = sl[(rr == r) & (dd == d)]
                    npair = (len(js) + 1) // 2
                    if npair == 0:
                        continue
                    q0 = counters[r, c, d]
                    counters[r, c, d] += npair
                    # element slots m0..m0+npair-1 (linear over subrange)
                    mm = m0 + np.arange(npair)
                    m0 += npair
                    kq = mm // P
                    pp = mm % P
                    # element index value (pair units, whole ring buffer)
                    ev = d * B_half + (q0 + np.arange(npair))
                    # element slot within the *section* for eidx layout
                    kq_sec = kq + (base_chunk - sec_chunk) // 2
                    esl = (off[b] * P + sec0[b, r] * P) // 2 + kq_sec * P + pp
                    eidx[c, esl] = ev.astype(np.int32)
                    # positions of the two rows of each pair
                    t1 = (base_chunk + 2 * kq) * P + pp
                    t2 = (base_chunk + 2 * kq + 1) * P + pp
                    bq = 2 * (q0 + np.arange(npair))       # within-bucket row
                    j1 = js[0::2]
                    j2 = js[1::2]
                    posslot[c, t1[:len(j1)]] = j1
                    pos_of_slot[j1] = t1[:len(j1)]
                    sidx_lin[c, t1] = d * B_pad + bq
                    recvrow_of_slot[tau[j1]] = c * B_pad + bq[:len(j1)]
                    posslot[c, t2[:len(j2)]] = j2
                    pos_of_slot[j2] = t2[:len(j2)]
                    sidx_lin[c, t2] = d * B_pad + bq + 1
                    recvrow_of_slot[tau[j2]] = c * B_pad + bq[:len(j2)] + 1
                assert m0 <= nch_i * P // 2, (c, g, r, m0, nch_i)
    assert (pos_of_slot[core_of_slot >= 0] >= 0).all()
    # rows of trash/unused positions target the own-core trash rows
    sidx_lin[sidx_lin < 0] = 0  # unused by emulate (only real positions read)
    for c in range(NCORES):
        real = posslot[c] >= 0
        sidx_lin[c, ~real] = (c + 1) * B_pad - 1
    gidx_lin = np.zeros((NCORES, TC), dtype=np.int32)
    for c in range(NCORES):
        real = np.where(posslot[c] >= 0)[0]
        gidx_lin[c, real] = recvrow_of_slot[posslot[c][real]].astype(np.int32)
    assert gidx_lin.max() < 8 * B_pad and gidx_lin.min() >= 0

    # S [128, TC] fp8 (col = chunk*128 + node-in-bin), E [128, TC] fp8
    # (partition = node-in-bin, col = position); eaT [BOND_F, TC]
    S = np.zeros((NCORES, P, TC), dtype=np.float32)
    Em = np.zeros((NCORES, P, TC), dtype=np.float32)
    eaT = np.zeros((NCORES, BOND_F, TC), dtype=np.float32)
    for c in range(NCORES):
        real = np.where(posslot[c] >= 0)[0]
        js = posslot[c][real]
        nn = nidx_of_slot[js]
        pp = real % P
        kk = real // P
        S[c][pp, kk * P + nn] = 1.0
        Em[c][nn, real] = 1.0
        f = rev[sigma[js]]
        eaT[c][:, real] = edge_attr[f].T

    # chunk -> bin-in-block map and E-matmul segments per 512 window
    kbin = []          # per block: list (len CB[b]) of bin-in-block or -1 pad
    for b in range(NB):
        kb = -np.ones(CB[b], dtype=np.int64)
        for r in range(NRING):
            for i in range(BLK):
                a = substart[b, r, i]
                kb[a:a + sub[b, r, i]] = i
        kbin.append(kb)

    # idx tensors: 16-partition wrap replicated to 128 partitions
    def wrap16(lin):
        # lin [L] int -> [128, L//16] with tile[16m+p, s] = lin[s*16+p]
        L = len(lin)
        assert L % 16 == 0
        w = np.asarray(lin, dtype=np.int16).reshape(L // 16, 16).T  # [16, L/16]
        return np.tile(w, (8, 1))

    gidx16 = np.zeros((NCORES, P, TC // 16), dtype=np.int16)
    for c in range(NCORES):
        gidx16[c] = wrap16(gidx_lin[c])
    # scatter idx (pair-element granular): per (block, ring) section
    scol = np.zeros((NB, NRING), dtype=np.int64)   # col offset (in idx cols)
    sw = 0
    for b in range(NB):
        for r in range(NRING):
            scol[b, r] = sw
            sw += (sec1[b, r] - sec0[b, r]) * P // 32   # elems/16 per section
    sidx16 = np.zeros((NCORES, P, max(sw, 16)), dtype=np.int16)
    for c in range(NCORES):
        for b in range(NB):
            for r in range(NRING):
                e0 = (off[b] + sec0[b, r]) * P // 2
                e1 = (off[b] + sec1[b, r]) * P // 2
                if e1 > e0:
                    w = wrap16(eidx[c, e0:e1])
                    sidx16[c][:, scol[b, r]:scol[b, r] + (e1 - e0) // 16] = w

    # node-level arrays (padded per bin)
    x_pad = np.zeros((NCORES, G * P, NODE_F), dtype=np.float32)
    for c in range(NCORES):
        for g, (n0, ncnt, _, _) in enumerate(bins[c]):
            if ncnt:
                x_pad[c, g * P: g * P + ncnt] = x[n0:n0 + ncnt]
    W_i_x = W_i[:, :NODE_F]
    xW_pad = np.einsum("cnf,hf->cnh", x_pad, W_i_x).astype(np.float32)

    meta = dict(N=N, E=E, G=G, NB=NB, B_pad=B_pad, TC=TC, bins=bins,
                CB=CB.tolist(), off=off.tolist(), rb=rb.tolist(),
                sub=sub, substart=substart, sec0=sec0, sec1=sec1,
                scol=scol, kbin=kbin, sw=sw,
                posslot=posslot, sidx_lin=sidx_lin, gidx_lin=gidx_lin,
                rho=rho_of_slot, tau=tau)
    percore = []
    for c in range(NCORES):
        percore.append({
            "gidx16": gidx16[c],
            "sidx16": sidx16[c] if sw else np.zeros((P, 16), np.int16),
            "S": S[c].astype(FP8),
            "E": Em[c].astype(FP8),
            "eaT": eaT[c].astype(BF16),
            "x_pad": x_pad[c].astype(np.float32),
            "xT_pad": x_pad[c].T.copy().astype(BF16),
            "xW_pad": xW_pad[c].astype(BF16),
            "WieT": W_i[:, NODE_F:].T.copy().astype(BF16),
            "WhT": W_h.T.copy().astype(BF16),
            "WoxT": W_o[:, :NODE_F].T.copy().astype(BF16),
            "WoMT": W_o[:, NODE_F:].T.copy().astype(BF16),
            "negI": (-np.eye(P)).astype(BF16),
            "Ident": np.eye(P).astype(np.float32),
            "IdentB": np.eye(P).astype(BF16),
            "b_i": b_i.reshape(P, 1).astype(np.float32),
            "b_h": b_h.reshape(P, 1).astype(np.float32),
            "b_o_row": b_o.reshape(1, P).astype(BF16),
        })
    return meta, percore


# ----------------------------------------------------------------------------
# numpy emulation of the device pipeline (indexing validation)
# ----------------------------------------------------------------------------

def emulate(meta, percore, inputs):
    x = np.asarray(inputs["x"], np.float32)
    ea = np.asarray(inputs["edge_attr"], np.float32)
    W_i = np.asarray(inputs["W_i"], np.float32)
    b_i = np.asarray(inputs["b_i"], np.float32)
    W_h = np.asarray(inputs["W_h"], np.float32)
    b_h = np.asarray(inputs["b_h"], np.float32)
    W_o = np.asarray(inputs["W_o"], np.float32)
    b_o = np.asarray(inputs["b_o"], np.float32)
    src = np.asarray(inputs["edge_index"][0], np.int64)
    dst = np.asarray(inputs["edge_index"][1], np.int64)
    rev = np.asarray(inputs["rev_edge_index"], np.int64)

    TC, NB, B_pad, G = meta["TC"], meta["NB"], meta["B_pad"], meta["G"]
    posslot = meta["posslot"]
    sidx_lin, gidx_lin = meta["sidx_lin"], meta["gidx_lin"]
    rho = meta["rho"]
    off = meta["off"]
    rb = meta["rb"]
    bins = meta["bins"]

    sigma = np.argsort(dst, kind="stable")

    # per-core position arrays
    h0b = np.zeros((NCORES, TC, HID), np.float32)
    realm = posslot >= 0
    for c in range(NCORES):
        js = posslot[c][realm[c]]
        f = rev[sigma[js]]
        h0b[c][realm[c]] = (x[src[f]] @ W_i[:, :NODE_F].T
                            + ea[f] @ W_i[:, NODE_F:].T + b_i)
    H = np.maximum(h0b, 0.0)

    def ship(rows_out):
        # rows_out [NCORES, TC, HID] -> gathered rows [NCORES, TC, HID]
        send = np.zeros((NRING, NCORES, 8 * B_pad + 16, HID), np.float32)
        for c in range(NCORES):
            js = posslot[c][realm[c]]
            rr = rho[js]
            si = sidx_lin[c][realm[c]]
            send[rr, c, si] = rows_out[c][realm[c]]
        # A2A per ring
        recv = np.zeros((NRING, NCORES, 8 * B_pad, HID), np.float32)
        for r in range(NRING):
            for d in range(NCORES):
                for cc in range(NCORES):
                    recv[r, d, cc * B_pad:(cc + 1) * B_pad] = \
                        send[r, cc, d * B_pad:(d + 1) * B_pad]
        rows_in = np.zeros((NCORES, TC, HID), np.float32)
        for c in range(NCORES):
            for b in range(NB):
                t0, t1 = off[b] * P, off[b + 1] * P
                rows_in[c, t0:t1] = recv[rb[b], c][gidx_lin[c, t0:t1]]
        return rows_in

    def aggregate(rows_in):
        # agg per bin node from S; also M = agg[node(t)] - rows_in[t]
        agg = np.zeros((NCORES, G, P, HID), np.float32)
        for c in range(NCORES):
            js = posslot[c][realm[c]]
            g = np.zeros(len(js), np.int64)
            # bin of slot
            for gi, (n0, ncnt, s0, scnt) in enumerate(bins[c]):
                if scnt:
                    g[(js >= s0) & (js < s0 + scnt)] = gi
            nn = dst[sigma[js]] - np.array(
                [bins[c][gi][0] for gi in g])
            np.add.at(agg[c], (g, nn), rows_in[c][realm[c]])
        return agg

    for it in range(1, DEPTH):
        rows_out = np.zeros((NCORES, TC, HID), np.float32)
        rhs = W_h.T if it < DEPTH - 1 or True else None
        for c in range(NCORES):
            rows_out[c] = H[c] @ W_h.T
        rows_in = ship(rows_out)
        agg = aggregate(rows_in)
        for c in range(NCORES):
            js = posslot[c][realm[c]]
            g = np.zeros(len(js), np.int64)
            for gi, (n0, ncnt, s0, scnt) in enumerate(bins[c]):
                if scnt:
                    g[(js >= s0) & (js < s0 + scnt)] = gi
            nn = dst[sigma[js]] - np.array([bins[c][gi][0] for gi in g])
            M = agg[c][g, nn] - rows_in[c][realm[c]]
            Hc = np.zeros((TC, HID), np.float32)
            Hc[realm[c]] = np.maximum(h0b[c][realm[c]] + M + b_h, 0.0)
            H[c] = Hc

    # final aggregation of H itself (shipped with identity)
    rows_in = ship(H)
    agg = aggregate(rows_in)

    outs = []
    for c in range(NCORES):
        out = np.zeros((G * P, HID), np.float32)
        for gi, (n0, ncnt, s0, scnt) in enumerate(bins[c]):
            if not ncnt:
                continue
            a = agg[c][gi]
            mask = (a.sum(axis=1) == 0.0).astype(np.float32)[:, None]
            xg = np.zeros((P, NODE_F), np.float32)
            xg[:ncnt] = x[n0:n0 + ncnt]
            M = a + mask * xg
            o = np.concatenate([xg, M], axis=1) @ W_o.T + b_o
            out[gi * P:(gi + 1) * P] = np.maximum(o, 0.0)
        outs.append(out)
    return outs


def assemble(meta, outs):
    N = meta["N"]
    full = np.zeros((N, HID), np.float32)
    for c in range(NCORES):
        for g, (n0, ncnt, _, _) in enumerate(meta["bins"][c]):
            if ncnt:
                full[n0:n0 + ncnt] = outs[c][g * P: g * P + ncnt]
    return full


# ----------------------------------------------------------------------------
# bass kernel
# ----------------------------------------------------------------------------

def build_nc(meta):
    DT = mybir.dt
    NB, B_pad, TC, G = meta["NB"], meta["B_pad"], meta["TC"], meta["G"]
    CB, off, rb = meta["CB"], meta["off"], meta["rb"]
    sec0, sec1, scol, kbin = meta["sec0"], meta["sec1"], meta["scol"], meta["kbin"]
    sw = max(meta["sw"], 16)
    W = 8 * B_pad

    nc = bacc.Bacc("TRN2", target_bir_lowering=False, debug=False,
                   num_devices=NCORES,
                   num_swdge_queues=int(_os.environ.get("KNOB_NSWQ", "2")))
    t = {}
    def inp(name, shape, dt):
        t[name] = nc.dram_tensor(name, shape, dt, kind="ExternalInput")
        return t[name]

    inp("gidx16", [P, TC // 16], DT.int16)
    inp("sidx16", [P, sw], DT.int16)
    inp("S", [P, TC], DT.float8e4)
    inp("E", [P, TC], DT.float8e4)
    inp("eaT", [BOND_F, TC], DT.bfloat16)
    inp("x_pad", [G * P, NODE_F], DT.float32)
    inp("xT_pad", [P, G * P], DT.bfloat16)
    inp("xW_pad", [G * P, HID], DT.bfloat16)
    inp("WieT", [BOND_F, HID], DT.bfloat16)
    inp("WhT", [P, P], DT.bfloat16)
    inp("WoxT", [P, P], DT.bfloat16)
    inp("WoMT", [P, P], DT.bfloat16)
    inp("negI", [P, P], DT.bfloat16)
    inp("Ident", [P, P], DT.float32)
    inp("IdentB", [P, P], DT.bfloat16)
    inp("b_i", [P, 1], DT.float32)
    inp("b_h", [P, 1], DT.float32)
    inp("b_o_row", [1, P], DT.bfloat16)
    out_pad = nc.dram_tensor("out_pad", [G * P, HID], DT.float32,
                             kind="ExternalOutput")

    h0t = nc.dram_tensor("h0t", [P, TC], DT.bfloat16)
    # pair-element layout: row = 2 edge-rows (512B)
    sends = [[nc.dram_tensor(f"send{s}_{r}", [W // 2, 2 * HID], DT.bfloat16)
              for r in range(NRING)] for s in range(2)]
    recvs = [nc.dram_tensor(f"recv{r}", [W // 2, 2 * HID], DT.bfloat16)
             for r in range(NRING)]

    AF = mybir.ActivationFunctionType
    OP = mybir.AluOpType
    RG = [list(range(NCORES))]
    NOSC = bool(_os.environ.get("KNOB_NOSC"))
    NOGA = bool(_os.environ.get("KNOB_NOGA"))
    NOCC = bool(_os.environ.get("KNOB_NOCC"))
    NOZ = bool(_os.environ.get("KNOB_NOZ"))
    GSPLIT = int(_os.environ.get("KNOB_GSPLIT", "8"))
    SP = not _os.environ.get("KNOB_NOSP")

    with tile.TileContext(nc) as tc:
        with (
            tc.tile_pool(name="sb", bufs=2) as sb,
            tc.tile_pool(name="sm", bufs=int(_os.environ.get("KNOB_SMBUFS",
                                                             "3"))) as sm,
            tc.tile_pool(name="smh", bufs=2) as smh,
            tc.tile_pool(name="cst", bufs=1) as cst,
            tc.tile_pool(name="ps", bufs=2, space="PSUM") as ps,
            tc.tile_pool(name="ps1", bufs=2, space="PSUM") as ps1,
            tc.tile_pool(name="psA", bufs=2, space="PSUM") as psA,
        ):
            # resident constants
            c_wiet = cst.tile([BOND_F, HID], DT.bfloat16)
            nc.sync.dma_start(c_wiet[:], t["WieT"][:])
            c_wht = cst.tile([P, P], DT.bfloat16)
            nc.sync.dma_start(c_wht[:], t["WhT"][:])
            c_woxt = cst.tile([P, P], DT.bfloat16)
            nc.sync.dma_start(c_woxt[:], t["WoxT"][:])
            c_womt = cst.tile([P, P], DT.bfloat16)
            nc.sync.dma_start(c_womt[:], t["WoMT"][:])
            c_negi = cst.tile([P, P], DT.bfloat16)
            nc.sync.dma_start(c_negi[:], t["negI"][:])
            c_id = cst.tile([P, P], DT.float32)
            nc.sync.dma_start(c_id[:], t["Ident"][:])
            c_idb = cst.tile([P, P], DT.bfloat16)
            nc.sync.dma_start(c_idb[:], t["IdentB"][:])
            c_bi = cst.tile([P, 1], DT.float32)
            nc.sync.dma_start(c_bi[:], t["b_i"][:])
            c_bh = cst.tile([P, 1], DT.float32)
            nc.sync.dma_start(c_bh[:], t["b_h"][:])
            c_bibh = cst.tile([P, 1], DT.float32)
            nc.vector.tensor_tensor(c_bibh[:], c_bi[:], c_bh[:], op=OP.add)
            c_nbh = cst.tile([P, 1], DT.float32)
            nc.vector.tensor_scalar(c_nbh[:], c_bh[:], -1.0, None, op0=OP.mult)
            c_bo = cst.tile([1, P], DT.bfloat16)
            nc.sync.dma_start(c_bo[:], t["b_o_row"][:])
            c_one = cst.tile([1, P], DT.bfloat16)
            nc.vector.memset(c_one[:], 1.0)
            c_xt = cst.tile([P, G * P], DT.bfloat16)
            nc.sync.dma_start(c_xt[:], t["xT_pad"][:])
            c_zero = cst.tile([P, 2048], DT.bfloat16)
            nc.vector.memset(c_zero[:], 0.0)
            c_zb = c_zero[:, 0:P]
            c_si = cst.tile([P, sw], DT.int16)
            nc.scalar.dma_start(c_si[:], t["sidx16"][:, :])

            def zero_send(s, r):
                if NOZ:
                    return
                for z0 in range(0, W // 2, 1024):
                    z1 = min(z0 + 1024, W // 2)
                    k = (z1 - z0) // P
                    nc.scalar.dma_start(
                        sends[s][r].ap()[z0:z1, :]
                        .rearrange("(p k) h -> p (k h)", k=k),
                        c_zero[:, :k * 2 * P])

            def scatters(b, outrows, si_t, s):
                if NOSC:
                    return
                for r in range(NRING):
                    nchp = int(sec1[b, r] - sec0[b, r]) // 2   # pair-chunks
                    c0 = int(scol[b, r])
                    for k0 in range(0, nchp, GSPLIT):
                        k1 = min(k0 + GSPLIT, nchp)
                        L = (k1 - k0) * P
                        nc.gpsimd.dma_scatter_add(
                            sends[s][r][:],
                            outrows[:, (int(sec0[b, r]) + 2 * k0) * P
                                    :(int(sec0[b, r]) + 2 * k1) * P]
                            .rearrange("p (k h) -> p k h", h=2 * HID),
                            si_t[:, c0 + k0 * 8:c0 + k1 * 8],
                            L, L, 2 * HID, single_packet=SP)

            # initial zero of both send sets
            for s in range(2):
                for r in range(NRING):
                    zero_send(s, r)

            # ---------------- phase 0 ----------------
            for b in range(NB):
                nch = CB[b]
                cols = slice(off[b] * P, off[b + 1] * P)
                e_t = sm.tile([P, nch * P], DT.float8e4, tag="E")
                nc.scalar.dma_start(e_t[:], t["E"][:, cols])
                ea_t = sm.tile([BOND_F, nch * P], DT.bfloat16, tag="ea")
                nc.sync.dma_start(ea_t[:], t["eaT"][:, cols])
                xw = []
                for i in range(BLK):
                    g = b * BLK + i
                    xw_g = sm.tile([P, HID], DT.bfloat16, tag=f"xw{i}")
                    nc.sync.dma_start(xw_g[:], t["xW_pad"][g * P:(g + 1) * P, :])
                    xw.append(xw_g)
                h0s = sb.tile([P, nch * P], DT.bfloat16, tag="h0s")
                outrows = sb.tile([P, nch * P], DT.bfloat16, tag="or")
                kb = kbin[b]
                for s in range(nch // 4):
                    if all(int(kb[s * 4 + k]) < 0 for k in range(4)):
                        continue
                    win = slice(s * 512, (s + 1) * 512)
                    pm = ps.tile([P, 512], DT.float32, tag="pm")
                    binw = [max(int(kb[s * 4 + k]), 0) for k in range(4)]
                    if len(set(binw)) == 1:
                        nc.tensor.matmul(pm[:, :], lhsT=xw[binw[0]][:],
                                         rhs=e_t[:, win], start=True, stop=False)
                        nc.tensor.matmul(pm[:, :], lhsT=c_wiet[:],
                                         rhs=ea_t[:, win], start=False, stop=True)
                    else:
                        for k in range(4):
                            ka = s * 4 + k
                            kc = slice(k * P, (k + 1) * P)
                            kca = slice(ka * P, (ka + 1) * P)
                            nc.tensor.matmul(pm[:, kc], lhsT=xw[binw[k]][:],
                                             rhs=e_t[:, kca], start=True,
                                             stop=False)
                            nc.tensor.matmul(pm[:, kc], lhsT=c_wiet[:],
                                             rhs=ea_t[:, kca], start=False,
                                             stop=True)
                    nc.vector.tensor_scalar(h0s[:, win], pm[:], c_bibh[:], None,
                                            op0=OP.add)
                    h1 = sb.tile([P, 512], DT.bfloat16, tag="h1")
                    nc.scalar.activation(h1[:], h0s[:, win], AF.Relu,
                                         bias=c_nbh[:])
                    pw = ps1.tile([P, 512], DT.float32, tag="pw")
                    for k in range(4):
                        kc = slice(k * P, (k + 1) * P)
                        nc.tensor.matmul(pw[:, kc], lhsT=h1[:, kc],
                                         rhs=c_wht[:], start=True, stop=True)
                    nc.vector.tensor_scalar(outrows[:, win], pw[:], 0.0, None,
                                            op0=OP.add)
                nc.sync.dma_start(h0t[:, cols], h0s[:])
                scatters(b, outrows, c_si, 0)

            # ---------------- phases 1..DEPTH ----------------
            for it in range(1, DEPTH + 1):
                sprev = (it - 1) % 2
                scur = it % 2
                if not NOCC:
                    for r in range(NRING):
                        nc.gpsimd.collective_compute(
                            "AllToAll", OP.bypass, replica_groups=RG,
                            ins=[sends[sprev][r][:]],
                            outs=[recvs[r][:]])
                if it == 2:
                    for r in range(NRING):
                        zero_send(0, r)
                last = it == DEPTH
                for b in range(NB):
                    nch = CB[b]
                    cols = slice(off[b] * P, off[b + 1] * P)
                    kb = kbin[b]
                    gi_t = sm.tile([P, nch * 8], DT.int16, tag="gi")
                    nc.sync.dma_start(gi_t[:],
                                      t["gidx16"][:, off[b] * 8:off[b + 1] * 8])
                    rows = sb.tile([P, nch * P], DT.bfloat16, tag="rows")
                    rnch = int(sec1[b, NRING - 1])  # skip window-pad chunks
                    if NOGA:
                        nc.vector.memset(rows[:], 0.0)
                    else:
                        for c0 in range(0, rnch, GSPLIT):
                            c1 = min(c0 + GSPLIT, rnch)
                            nc.gpsimd.dma_gather(
                                rows[:, c0 * P:c1 * P]
                                .rearrange("p (k h) -> p k h", h=HID),
                                recvs[rb[b]].ap()
                                .rearrange("w (two h) -> (w two) h", two=2),
                                gi_t[:, c0 * 8:c1 * 8],
                                (c1 - c0) * P, (c1 - c0) * P, HID,
                                single_packet=SP,
                                queue_num=1 if nc.num_swdge_queues > 1 else 0)
                    s_t = sm.tile([P, nch * P], DT.float8e4, tag="S")
                    nc.sync.dma_start(s_t[:], t["S"][:, cols])
                    # one PSUM bank (512 f32) per bin: start=True clears the
                    # has_written bits of the whole bank, so the two bins'
                    # accumulation chains must not share one
                    pa = psA.tile([P, BLK * 512], DT.float32, tag="pa")
                    pav = [pa[:, i * 512:i * 512 + HID] for i in range(BLK)]
                    kfirst = [min([k for k in range(nch) if kb[k] == i],
                                  default=-1) for i in range(BLK)]
                    klast = [max([k for k in range(nch) if kb[k] == i],
                                 default=-1) for i in range(BLK)]
                    for k in range(nch):
                        i = int(kb[k])
                        if i < 0:
                            continue
                        kc = slice(k * P, (k + 1) * P)
                        nc.tensor.matmul(pav[i],
                                         lhsT=s_t[:, kc], rhs=rows[:, kc],
                                         start=(k == kfirst[i]),
                                         stop=(k == klast[i]))
                    for i in range(BLK):
                        if kfirst[i] < 0:
                            nc.tensor.matmul(pav[i],
                                             lhsT=c_zb[:], rhs=rows[:, 0:P],
                                             start=True, stop=True)

                    if last:
                        # ---- readout ----
                        for i in range(BLK):
                            g = b * BLK + i
                            agg3 = sb.tile([P, HID], DT.float32, tag="agg3")
                            nc.scalar.activation(
                                agg3[:], pav[i], AF.Copy)
                            rsum = sb.tile([P, 1], DT.float32, tag="rsum")
                            nc.vector.tensor_reduce(rsum[:], agg3[:],
                                                    axis=mybir.AxisListType.X,
                                                    op=OP.add)
                            mask = sb.tile([P, 1], DT.float32, tag="mask")
                            nc.vector.tensor_scalar(mask[:], rsum[:], 0.0, None,
                                                    op0=OP.is_equal)
                            x_g = sb.tile([P, NODE_F], DT.float32, tag="xg")
                            nc.sync.dma_start(x_g[:],
                                              t["x_pad"][g * P:(g + 1) * P, :])
                            mx = sb.tile([P, NODE_F], DT.float32, tag="mx")
                            nc.vector.tensor_scalar(mx[:], x_g[:], mask[:],
                                                    None, op0=OP.mult)
                            Mg = sb.tile([P, HID], DT.float32, tag="Mg")
                            nc.vector.tensor_tensor(Mg[:], agg3[:], mx[:],
                                                    op=OP.add)
                            pt = ps1.tile([P, P], DT.float32, tag="pw")
                            nc.tensor.transpose(pt[:], Mg[:], c_id[:])
                            MgT = sb.tile([P, P], DT.bfloat16, tag="MgT")
                            nc.scalar.activation(MgT[:], pt[:], AF.Copy)
                            po = ps.tile([P, HID], DT.float32, tag="pm")
                            nc.tensor.matmul(
                                po[:], lhsT=c_xt[:, g * P:(g + 1) * P],
                                rhs=c_woxt[:], start=True, stop=False)
                            nc.tensor.matmul(po[:], lhsT=MgT[:], rhs=c_womt[:],
                                             start=False, stop=False)
                            nc.tensor.matmul(po[:], lhsT=c_one[:], rhs=c_bo[:],
                                             start=False, stop=True)
                            og = sb.tile([P, HID], DT.float32, tag="og")
                            nc.scalar.activation(og[:], po[:], AF.Relu)
                            nc.sync.dma_start(out_pad[g * P:(g + 1) * P, :],
                                              og[:])
                        continue

                    aggw = sb.tile([P, BLK * HID], DT.bfloat16, tag="aggw")
                    for i in range(BLK):
                        nc.scalar.activation(
                            aggw[:, i * HID:(i + 1) * HID], pav[i], AF.Copy)
                    e_t = sm.tile([P, nch * P], DT.float8e4, tag="E")
                    nc.scalar.dma_start(e_t[:], t["E"][:, cols])
                    h0_t = smh.tile([P, nch * P], DT.bfloat16, tag="h0g")
                    nc.sync.dma_start(h0_t[:], h0t[:, cols])
                    outrows = sb.tile([P, nch * P], DT.bfloat16, tag="or")
                    rhs_w = c_wht if it < DEPTH - 1 else c_idb
                    for s in range(nch // 4):
                        if all(int(kb[s * 4 + k]) < 0 for k in range(4)):
                            continue
                        win = slice(s * 512, (s + 1) * 512)
                        pm = ps.tile([P, 512], DT.float32, tag="pm")
                        binw = [max(int(kb[s * 4 + k]), 0) for k in range(4)]
                        if len(set(binw)) == 1:
                            nc.tensor.matmul(
                                pm[:, :],
                                lhsT=aggw[:, binw[0] * HID
                                          :(binw[0] + 1) * HID],
                                rhs=e_t[:, win], start=True, stop=False)
                            for k in range(4):
                                ka = s * 4 + k
                                kc = slice(k * P, (k + 1) * P)
                                kca = slice(ka * P, (ka + 1) * P)
                                nc.tensor.matmul(pm[:, kc], lhsT=rows[:, kca],
                                                 rhs=c_negi[:], start=False,
                                                 stop=True)
                        else:
                            for k in range(4):
                                ka = s * 4 + k
                                kc = slice(k * P, (k + 1) * P)
                                kca = slice(ka * P, (ka + 1) * P)
                                nc.tensor.matmul(
                                    pm[:, kc],
                                    lhsT=aggw[:, binw[k] * HID
                                              :(binw[k] + 1) * HID],
                                    rhs=e_t[:, kca], start=True, stop=False)
                                nc.tensor.matmul(pm[:, kc], lhsT=rows[:, kca],
                                                 rhs=c_negi[:], start=False,
                                                 stop=True)
                        tmp = sb.tile([P, 512], DT.bfloat16, tag="tmp")
                        nc.vector.tensor_tensor(tmp[:], pm[:], h0_t[:, win],
                                                op=OP.add)
                        hn = sb.tile([P, 512], DT.bfloat16, tag="hn")
                        nc.scalar.activation(hn[:], tmp[:], AF.Relu)
                        pw = ps1.tile([P, 512], DT.float32, tag="pw")
                        for k in range(4):
                            kc = slice(k * P, (k + 1) * P)
                            nc.tensor.matmul(pw[:, kc], lhsT=hn[:, kc],
                                             rhs=rhs_w[:], start=True,
                                             stop=True)
                        nc.vector.tensor_scalar(outrows[:, win], pw[:], 0.0,
                                                None, op0=OP.add)
                    scatters(b, outrows, c_si, scur)

    nc.compile()
    return nc


# ----------------------------------------------------------------------------
# PJRT SPMD runner (inlined; based on concourse.bass2jax.run_bass_via_pjrt)
# ----------------------------------------------------------------------------

class SpmdRunner:
    def __init__(self, nc, n_cores):
        import jax
        from jax.sharding import Mesh, PartitionSpec
        from jax.experimental.shard_map import shard_map
        from concourse.bass2jax import (
            _bass_exec_p, partition_id_tensor, install_neuronx_cc_hook)
        install_neuronx_cc_hook()
        self.jax = jax
        self.n_cores = n_cores
        in_names, out_names, out_avals, zero_outs = [], [], [], []
        partition_name = (
            nc.partition_id_tensor.name if nc.partition_id_tensor else None)
        for alloc in nc.m.functions[0].allocations:
            if not isinstance(alloc, mybir.MemoryLocationSet):
                continue
            name = alloc.memorylocations[0].name
            if alloc.kind == "ExternalInput":
                if name != partition_name:
                    in_names.append(name)
            elif alloc.kind == "ExternalOutput":
                out_names.append(name)
                shape = tuple(alloc.tensor_shape)
                dtype = mybir.dt.np(alloc.dtype)
                out_avals.append(jax.core.ShapedArray(shape, dtype))
                zero_outs.append(np.zeros(shape, dtype))
        self.in_names, self.out_names = in_names, out_names
        self.out_avals, self.zero_outs = out_avals, zero_outs
        n_params, n_outs = len(in_names), len(out_avals)
        all_in = list(in_names) + list(out_names)
        if partition_name is not None:
            all_in.append(partition_name)

        def _body(*args):
            operands = list(args)
            if partition_name is not None:
                operands.append(partition_id_tensor())
            return tuple(_bass_exec_p.bind(
                *operands, out_avals=tuple(out_avals),
                in_names=tuple(all_in), out_names=tuple(out_names),
                lowering_input_output_aliases=(),
                sim_require_finite=True, sim_require_nnan=True, nc=nc))

        devices = jax.devices()[:n_cores]
        assert len(devices) == n_cores
        self.mesh = Mesh(np.asarray(devices), ("core",))
        self.PartitionSpec = PartitionSpec
        in_specs = (PartitionSpec("core"),) * (n_params + n_outs)
        out_specs = (PartitionSpec("core"),) * len(out_names)
        self.fn = jax.jit(
            shard_map(_body, mesh=self.mesh, in_specs=in_specs,
                      out_specs=out_specs, check_rep=False),
            keep_unused=True)

    def pack(self, in_maps):
        n = self.n_cores
        concat = [np.concatenate(
            [np.asarray(in_maps[c][name]) for c in range(n)], axis=0)
            for name in self.in_names]
        for z in self.zero_outs:
            concat.append(np.zeros((n * z.shape[0], *z.shape[1:]), z.dtype))
        sharding = self.jax.sharding.NamedSharding(
            self.mesh, self.PartitionSpec("core"))
        return [self.jax.device_put(a, sharding) for a in concat]

    def run(self, packed):
        outs = self.fn(*packed)
        self.jax.block_until_ready(outs)
        return outs

    def unpack(self, outs):
        n = self.n_cores
        return [
            {name: np.asarray(outs[i]).reshape(n, *self.out_avals[i].shape)[c]
             for i, name in enumerate(self.out_names)}
            for c in range(n)]

    def time_exec(self, packed, iters=10, warmup=2):
        import time
        for _ in range(warmup):
            self.jax.block_until_ready(self.fn(*packed))
        t0 = time.perf_counter()
        outs = None
        for _ in range(iters):
            outs = self.fn(*packed)
        self.jax.block_until_ready(outs)
        t1 = time.perf_counter()
        return (t1 - t0) / iters, outs


def kernel(**inputs):
    inputs = {k: np.asarray(v) for k, v in inputs.items()}
    meta, percore = prep(**inputs)
    nc = build_nc(meta)
    r = SpmdRunner(nc, NCORES)
    packed = r.pack(percore)
    outs = r.run(packed)
    res = r.unpack(outs)
    return assemble(meta, [res[c]["out_pad"] for c in range(NCORES)])

